# revision 1
# baseline (speedup 1.0000x reference)
"""Dual-stream attention kernel for Trainium2 (8 NeuronCores, SPMD).

Problem: B=4, S=4096, DIM=256
  out1 = LN(mean(x1,1) + softmax(mask(sum_j tanh(k1 @ q2.T))) @ v1)
  out2 = LN(mean(x2,1) + softmax(mask(sum_j tanh(k2 @ q1.T))) @ v2)

Sharding: 8 independent (batch, stream) units -> one per core, no
cross-core communication. Core 2*b+s handles batch b, stream s.
"""

import numpy as np

B, S, DIM = 4, 4096, 256
P = 128
MB = DIM // P      # 2 d-blocks of 128 partitions
SB = S // P        # 32 seq blocks of 128
JC = 512           # score j-chunk (one PSUM bank of fp32)
NJ = S // JC       # 8 chunks
EPS = 1e-5
NCORES = 8

_PROG = {}         # cached Bass programs by reps (compiled once per process)


def _build_program(reps=1):
    import concourse.bacc as bacc
    import concourse.tile as tile
    from concourse import mybir

    f32 = mybir.dt.float32
    f32r = mybir.dt.float32r
    AF = mybir.ActivationFunctionType
    AX = mybir.AxisListType
    OP = mybir.AluOpType

    nc = bacc.Bacc("TRN2", target_bir_lowering=False, debug=False)

    # ---- DRAM I/O (per-core data; weights replicated) ----
    xaT_d = nc.declare_dram_parameter("xaT", [DIM, S], f32r, False)
    xbT_d = nc.declare_dram_parameter("xbT", [DIM, S], f32r, False)
    wkT_d = nc.declare_dram_parameter("wkT", [DIM, DIM], f32r, False)
    wqT_d = nc.declare_dram_parameter("wqT", [DIM, DIM], f32r, False)
    wvT_d = nc.declare_dram_parameter("wvT", [DIM, DIM], f32r, False)
    bk_d = nc.declare_dram_parameter("bk", [P, MB], f32, False)
    bq_d = nc.declare_dram_parameter("bq", [P, MB], f32, False)
    bv_d = nc.declare_dram_parameter("bv", [1, DIM], f32r, False)
    madd_d = nc.declare_dram_parameter("madd", [P, SB], f32, False)
    gamma_d = nc.declare_dram_parameter("gamma", [P, MB], f32, False)
    beta_d = nc.declare_dram_parameter("beta", [P, MB], f32, False)
    sel_d = nc.declare_dram_parameter("sel", [4, 2], f32, False)
    onesr_d = nc.declare_dram_parameter("onesr", [1, P], f32r, False)
    out_d = nc.declare_dram_parameter("out", [P, MB], f32, True)

    with tile.TileContext(nc) as tc:
        with (
            tc.tile_pool(name="const", bufs=1) as const,
            tc.tile_pool(name="big", bufs=1) as big,
            tc.tile_pool(name="work", bufs=2) as work,
            tc.tile_pool(name="csums", bufs=4) as csums_pool,
            tc.tile_pool(name="mmps", bufs=2, space="PSUM") as mm_psum,
        ):
            vp_psum = mm_psum   # v-proj tiles share the mm slots
            vec_psum = mm_psum  # tail psum tiles share the mm slots too
            # ---- load constants/weights ----
            wk = [const.tile([P, DIM], f32r, tag=f"wk{k}", name=f"wk{k}") for k in range(MB)]
            wq = [const.tile([P, DIM], f32r, tag=f"wq{k}", name=f"wq{k}") for k in range(MB)]
            wv = [const.tile([P, DIM], f32r, tag=f"wv{k}", name=f"wv{k}") for k in range(MB)]
            bk_sb = const.tile([P, MB], f32, tag="bk")
            bq_sb = const.tile([P, MB], f32, tag="bq")
            bv_sb = const.tile([1, DIM], f32r, tag="bv")
            madd_sb = const.tile([P, SB], f32, tag="madd")
            gamma_sb = const.tile([P, MB], f32, tag="gamma")
            beta_sb = const.tile([P, MB], f32, tag="beta")
            nc.sync.dma_start(out=bk_sb, in_=bk_d[:, :])
            nc.sync.dma_start(out=bq_sb, in_=bq_d[:, :])
            nc.sync.dma_start(out=bv_sb, in_=bv_d[:, :])
            nc.sync.dma_start(out=madd_sb, in_=madd_d[:, :])
            nc.sync.dma_start(out=gamma_sb, in_=gamma_d[:, :])
            nc.sync.dma_start(out=beta_sb, in_=beta_d[:, :])
            ones_row = const.tile([1, P], f32, tag="ones")
            nc.gpsimd.memset(ones_row, 1.0)
            ones_row_r = const.tile([1, P], f32r, tag="onesr")
            nc.sync.dma_start(out=ones_row_r, in_=onesr_d[:, :])
            ones_col = const.tile([P, 1], f32, tag="onesc")
            nc.gpsimd.memset(ones_col, 1.0)
            eps_sb = const.tile([P, 1], f32, tag="eps")
            nc.gpsimd.memset(eps_sb, EPS)
            negs_sb = const.tile([P, 1], f32, tag="negs")
            nc.gpsimd.memset(negs_sb, -float(S))
            sel_sb = const.tile([4, 2], f32, tag="sel")
            nc.sync.dma_start(out=sel_sb, in_=sel_d[:, :])

            for rep in range(reps):
                xa = [big.tile([P, S], f32r, tag=f"xa{k}", name=f"xa{k}") for k in range(MB)]
                xb = [big.tile([P, S], f32r, tag=f"xb{k}", name=f"xb{k}") for k in range(MB)]
                # issue order: k-proj inputs first so PE starts ASAP
                for k in range(MB):
                    nc.sync.dma_start(out=wk[k], in_=wkT_d[k * P:(k + 1) * P, :])
                for k in range(MB):
                    for c in range(4):
                        nc.sync.dma_start(
                            out=xa[k][:, c * 1024:(c + 1) * 1024],
                            in_=xaT_d[k * P:(k + 1) * P,
                                      c * 1024:(c + 1) * 1024])
                for k in range(MB):
                    nc.sync.dma_start(out=wq[k], in_=wqT_d[k * P:(k + 1) * P, :])
                for k in range(MB):
                    for c in range(2):
                        nc.sync.dma_start(
                            out=xb[k][:, c * 2048:(c + 1) * 2048],
                            in_=xbT_d[k * P:(k + 1) * P,
                                      c * 2048:(c + 1) * 2048])
                for k in range(MB):
                    nc.sync.dma_start(out=wv[k], in_=wvT_d[k * P:(k + 1) * P, :])
                # ---- row-sum of xa (for mean over seq) ----
                xsum = work.tile([P, MB], f32, tag="xsum")
                for k in range(MB):
                    nc.vector.reduce_sum(out=xsum[:, k:k + 1], in_=xa[k], axis=AX.X)

                # ---- projections k,q in [d, s] layout (relu+bias on DVE) ----
                kt = [big.tile([P, S], f32r, tag=f"kt{k}", name=f"kt{k}") for k in range(MB)]
                qt = [big.tile([P, S], f32r, tag=f"qt{k}", name=f"qt{k}") for k in range(MB)]
                for dst, wsb, bsb, src in ((kt, wk, bk_sb, xa), (qt, wq, bq_sb, xb)):
                    for m in range(MB):
                        for ng in range(2):  # [128,2048] psum tiles (4 banks)
                            ps = mm_psum.tile([P, 4 * JC], f32, tag="mm",
                                              name=f"kqps{ng}")
                            for kk in range(MB):
                                for h in range(4):
                                    n = ng * 4 + h
                                    nc.tensor.matmul(
                                        ps[:, h * JC:(h + 1) * JC],
                                        lhsT=wsb[kk][:, m * P:(m + 1) * P],
                                        rhs=src[kk][:, n * JC:(n + 1) * JC],
                                        start=(kk == 0), stop=(kk == MB - 1),
                                    )
                            # relu(psum + bias) on ACT (idle pre-scores)
                            nc.scalar.activation(
                                out=dst[m][:, ng * 4 * JC:(ng + 1) * 4 * JC],
                                in_=ps, func=AF.Relu,
                                bias=bsb[:, m:m + 1],
                            )

                # ---- projection v in natural layout [s, d] ----
                # bias broadcast along partitions via a K=1 ones matmul
                v_sb = big.tile([P, SB, DIM], f32, tag="v")
                for si in range(SB):
                    ps = vp_psum.tile([P, DIM], f32, tag="mm", name="vps")
                    for kk in range(MB):
                        nc.tensor.matmul(
                            ps,
                            lhsT=xa[kk][:, si * P:(si + 1) * P],
                            rhs=wv[kk],
                            start=(kk == 0), stop=False,
                        )
                    nc.tensor.matmul(
                        ps, lhsT=ones_row_r, rhs=bv_sb,
                        start=False, stop=True,
                    )
                    nc.vector.tensor_scalar_max(
                        out=v_sb[:, si, :], in0=ps, scalar1=0.0)

                # ---- scores + streaming softmax numerator ----
                # s[i] = sum_j tanh(k[i].q[j]); since s <= S the constant
                # shift -S is a safe softmax stabilizer, so each i-block's
                # exp(s - S) and its e*v contribution stream during the
                # score phase (no global max pass, no tail vec matmuls).
                s_sb = work.tile([P, SB], f32, tag="s")
                e_sb = work.tile([P, SB], f32, tag="e")
                zcols = work.tile([P, 6], f32, tag="zc")
                vacc = work.tile([P, DIM], f32, tag="vacc")
                nc.vector.memset(vacc, 0.0)
                for ib in range(SB):
                    cs = csums_pool.tile([P, 2], f32, tag="cs")
                    for jg in range(2):  # [128,2048] psum tiles (4 banks)
                        ps = mm_psum.tile([P, 4 * JC], f32, tag="mm",
                                          name=f"scps{jg}")
                        for kk in range(MB):
                            for h in range(4):
                                n = jg * 4 + h
                                nc.tensor.matmul(
                                    ps[:, h * JC:(h + 1) * JC],
                                    lhsT=kt[kk][:, ib * P:(ib + 1) * P],
                                    rhs=qt[kk][:, n * JC:(n + 1) * JC],
                                    start=(kk == 0), stop=(kk == MB - 1),
                                )
                        # tanh in place in PSUM; row-sum via accum_out
                        nc.scalar.activation(
                            out=ps, in_=ps, func=AF.Tanh,
                            accum_out=cs[:, jg:jg + 1],
                        )
                    # masked score column: s = rowsum + madd
                    nc.vector.reduce_sum(out=s_sb[:, ib:ib + 1], in_=cs,
                                         axis=AX.X)
                    nc.vector.tensor_add(
                        s_sb[:, ib:ib + 1], s_sb[:, ib:ib + 1],
                        madd_sb[:, ib:ib + 1])
                    ends = {7: (0, 0, 8), 15: (1, 8, 16), 23: (2, 16, 24),
                            27: (3, 24, 28), 29: (4, 28, 30), 31: (5, 30, 32)}
                    if ib in ends:
                        g, lo, hi = ends[ib]
                        # exp over a group of masked columns (smaller final
                        # groups shorten the tail accumulation chain)
                        nc.scalar.activation(
                            out=e_sb[:, lo:hi],
                            in_=s_sb[:, lo:hi],
                            func=AF.Exp, bias=negs_sb,
                            accum_out=zcols[:, g:g + 1])
                        for b8 in range(lo, hi):
                            # vacc += e[block] * v[block]
                            nc.vector.scalar_tensor_tensor(
                                out=vacc, in0=v_sb[:, b8, :],
                                scalar=e_sb[:, b8:b8 + 1], in1=vacc,
                                op0=OP.mult, op1=OP.add)

                # ---- softmax denominator ----
                zp = work.tile([P, 1], f32, tag="zp")
                nc.vector.reduce_sum(out=zp, in_=zcols, axis=AX.X)
                z_ps = vec_psum.tile([1, 1], f32, tag="mm", name="z_ps")
                nc.tensor.matmul(z_ps, lhsT=zp,
                                 rhs=ones_col, start=True, stop=True)
                z_sb = work.tile([1, 1], f32, tag="z1")
                nc.vector.tensor_copy(out=z_sb, in_=z_ps)
                invz1 = work.tile([1, 1], f32, tag="invz1")
                nc.vector.reciprocal(out=invz1, in_=z_sb)
                invz_ps = vec_psum.tile([P, 1], f32, tag="mm", name="invz_ps")
                nc.tensor.matmul(invz_ps, lhsT=ones_row,
                                 rhs=invz1, start=True, stop=True)
                invz = work.tile([P, 1], f32, tag="invz")
                nc.vector.tensor_copy(out=invz, in_=invz_ps)

                # ---- vec: partition-sum the streamed accumulator ----
                vecp = vec_psum.tile([P, MB], f32, tag="mm", name="vecp")
                for m in range(MB):
                    nc.tensor.matmul(
                        vecp[:, m:m + 1],
                        lhsT=vacc[:, m * P:(m + 1) * P],
                        rhs=ones_col, start=True, stop=True)

                # ---- y = mean(xa) + vec/Z ----
                vscaled = work.tile([P, MB], f32, tag="vs")
                nc.vector.tensor_scalar_mul(out=vscaled, in0=vecp, scalar1=invz)
                # stat4 cols: [y0, y1, y0^2, y1^2]; y_sb aliases cols 0:2
                stat4 = work.tile([P, 4], f32, tag="stat4")
                y_sb = stat4[:, 0:MB]
                nc.vector.scalar_tensor_tensor(
                    out=y_sb, in0=xsum, scalar=1.0 / S, in1=vscaled,
                    op0=OP.mult, op1=OP.add)

                # ---- layernorm over d=256 (spans 2 partition blocks) ----
                nc.vector.tensor_mul(stat4[:, MB:2 * MB], y_sb, y_sb)
                r4_ps = vec_psum.tile([4, 1], f32, tag="mm", name="r4_ps")
                nc.tensor.matmul(r4_ps, lhsT=stat4,
                                 rhs=ones_col, start=True, stop=True)
                r4 = work.tile([4, 1], f32, tag="r4")
                nc.vector.tensor_copy(out=r4, in_=r4_ps)
                s12_ps = vec_psum.tile([1, 2], f32, tag="mm", name="s12_ps")
                nc.tensor.matmul(s12_ps, lhsT=r4,
                                 rhs=sel_sb, start=True, stop=True)
                s12 = work.tile([1, 2], f32, tag="s12")
                nc.vector.tensor_copy(out=s12, in_=s12_ps)
                # mu = sum(y)/D ; ex2 = sum(y^2)/D ; var = ex2 - mu^2
                ms = work.tile([1, 2], f32, tag="ms")
                nc.vector.tensor_scalar_mul(out=ms, in0=s12,
                                            scalar1=1.0 / DIM)
                mu2 = work.tile([1, 1], f32, tag="mu2")
                nc.vector.tensor_mul(mu2, ms[:, 0:1], ms[:, 0:1])
                var = work.tile([1, 1], f32, tag="var")
                nc.vector.tensor_sub(var, ms[:, 1:2], mu2)
                # rstd = exp(-0.5*ln(var+eps))  (ln/exp share a table set)
                lnv = work.tile([1, 1], f32, tag="lnv")
                nc.scalar.activation(out=lnv, in_=var, func=AF.Ln,
                                     bias=eps_sb[0:1, :])
                mr1 = work.tile([1, 2], f32, tag="mr1")
                nc.vector.tensor_copy(out=mr1[:, 0:1], in_=ms[:, 0:1])
                nc.scalar.activation(out=mr1[:, 1:2], in_=lnv, func=AF.Exp,
                                     scale=-0.5)
                # broadcast [mu, rstd] to all partitions
                mr_ps = vec_psum.tile([P, 2], f32, tag="mm", name="mr_ps")
                nc.tensor.matmul(mr_ps, lhsT=ones_row,
                                 rhs=mr1, start=True, stop=True)
                mr_sb = work.tile([P, 2], f32, tag="mr")
                nc.vector.tensor_copy(out=mr_sb, in_=mr_ps)
                # (y - mu) * rstd
                norm = work.tile([P, MB], f32, tag="norm")
                nc.vector.tensor_scalar(
                    out=norm, in0=y_sb, scalar1=mr_sb[:, 0:1],
                    scalar2=mr_sb[:, 1:2], op0=OP.subtract, op1=OP.mult)
                normg = work.tile([P, MB], f32, tag="normg")
                nc.vector.tensor_mul(normg, norm, gamma_sb)
                out_sb = work.tile([P, MB], f32, tag="out")
                nc.vector.tensor_add(out_sb, normg, beta_sb)
                nc.sync.dma_start(out=out_d[:, :], in_=out_sb)

    nc.finalize()
    return nc


def _get_program(reps=1):
    if reps not in _PROG:
        _PROG[reps] = _build_program(reps)
    return _PROG[reps]


def _pn(v):
    """[DIM] -> [P, MB] with tile[p, m] = v[m*128 + p]."""
    return np.ascontiguousarray(np.asarray(v, np.float32).reshape(MB, P).T)


def make_in_maps(fingerprint_vectors1, fingerprint_vectors2, mask1, mask2,
                 Wq, bq, Wk, bk, Wv, bv, gamma, beta):
    x1 = np.asarray(fingerprint_vectors1, np.float32)
    x2 = np.asarray(fingerprint_vectors2, np.float32)
    m1 = np.asarray(mask1, bool)
    m2 = np.asarray(mask2, bool)
    x1T = np.ascontiguousarray(x1.transpose(0, 2, 1))  # [B, D, S]
    x2T = np.ascontiguousarray(x2.transpose(0, 2, 1))
    wqT = np.ascontiguousarray(np.asarray(Wq, np.float32).T)
    wkT = np.ascontiguousarray(np.asarray(Wk, np.float32).T)
    wvT = np.ascontiguousarray(np.asarray(Wv, np.float32).T)
    shared = {
        "wkT": wkT, "wqT": wqT, "wvT": wvT,
        "bk": _pn(bk), "bq": _pn(bq),
        "bv": np.ascontiguousarray(np.asarray(bv, np.float32).reshape(1, DIM)),
        "gamma": _pn(gamma), "beta": _pn(beta),
        "sel": np.array([[1, 0], [1, 0], [0, 1], [0, 1]], np.float32),
        "onesr": np.ones((1, P), np.float32),
    }
    in_maps = []
    for b in range(B):
        for stream in range(2):
            if stream == 0:
                xa, xbt, msk = x1T[b], x2T[b], m1[b]
            else:
                xa, xbt, msk = x2T[b], x1T[b], m2[b]
            madd = np.where(msk, np.float32(-1e30), np.float32(0.0))
            madd = np.ascontiguousarray(
                madd.astype(np.float32).reshape(SB, P).T)
            in_maps.append(dict(shared, xaT=xa, xbT=xbt, madd=madd))
    return in_maps


# test.py can flip these to get a profile out of the run
RUN_OPTS = {"trace": False, "trace_kwargs": None}
LAST = {}


def kernel(**inputs):
    from concourse.bass_utils import run_bass_kernel_spmd

    nc = _get_program()
    in_maps = make_in_maps(**inputs)
    kw = {}
    if RUN_OPTS.get("trace"):
        kw["trace"] = True
        if RUN_OPTS.get("trace_kwargs"):
            kw["trace_kwargs"] = RUN_OPTS["trace_kwargs"]
    res = run_bass_kernel_spmd(nc, in_maps, list(range(NCORES)), **kw)
    LAST["exec_time_ns"] = res.exec_time_ns
    LAST["profile_json"] = res.profile_json
    outs = res.results
    out1 = np.stack([np.asarray(outs[2 * b]["out"]).T.reshape(DIM)
                     for b in range(B)])
    out2 = np.stack([np.asarray(outs[2 * b + 1]["out"]).T.reshape(DIM)
                     for b in range(B)])
    return out1.astype(np.float32), out2.astype(np.float32)



# revision 2
# speedup vs baseline: 1.3692x; 1.3692x over previous
"""Dual-stream attention kernel for Trainium2 (8 NeuronCores, SPMD).

Problem: B=4, S=4096, DIM=256
  out1 = LN(mean(x1,1) + softmax(mask(sum_j tanh(k1 @ q2.T))) @ v1)
  out2 = LN(mean(x2,1) + softmax(mask(sum_j tanh(k2 @ q1.T))) @ v2)

Sharding: 8 independent (batch, stream) units -> one per core, no
cross-core communication. Core 2*b+s handles batch b, stream s.

Mask trimming: softmax rows i with mask=True get weight exactly 0, so
k/v/scores are only needed for unmasked rows. The host permutes each
core's own-stream x into [unmasked | masked] column order and the
device computes only the first NB*128 row slots (capacity ~2048+8
sigma for a random half mask); slots past the unmasked count are
killed by the -1e30 mask-add exactly like the full kernel would.
A full 32-block program is kept as a fallback if a mask ever exceeds
capacity.
"""

import numpy as np

B, S, DIM = 4, 4096, 256
P = 128
MB = DIM // P      # 2 d-blocks of 128 partitions
NB = 18            # row-slot capacity in blocks of 128 (C = 2304)
JC = 512           # score j-chunk (one PSUM bank of fp32)
EPS = 1e-5
NCORES = 8

_PROG = {}         # cached Bass programs by (reps, nb)


def _group_sizes(nb):
    # exp/e*v groups over the i-blocks; shrinking tail groups shorten
    # the final accumulation chain
    if nb == 18:
        return [6, 6, 3, 2, 1]
    if nb == 32:
        return [8, 8, 8, 4, 2, 2]
    sizes, rem = [], nb
    while rem > 0:
        g = max(1, min(rem // 2, 8)) if rem > 2 else rem
        sizes.append(g)
        rem -= g
    return sizes


def _build_program(reps=1, nb=NB):
    import concourse.bacc as bacc
    import concourse.tile as tile
    from concourse import mybir

    f32 = mybir.dt.float32
    f32r = mybir.dt.float32r
    AF = mybir.ActivationFunctionType
    AX = mybir.AxisListType
    OP = mybir.AluOpType

    C = nb * P
    sizes = _group_sizes(nb)
    ends = {}
    lo = 0
    for g, sz in enumerate(sizes):
        ends[lo + sz - 1] = (g, lo, lo + sz)
        lo += sz
    ngroups = len(sizes)

    nc = bacc.Bacc("TRN2", target_bir_lowering=False, debug=False)

    # ---- DRAM I/O (per-core data; weights replicated) ----
    xaT_d = nc.declare_dram_parameter("xaT", [DIM, S], f32r, False)
    xbT_d = nc.declare_dram_parameter("xbT", [DIM, S], f32r, False)
    wkT_d = nc.declare_dram_parameter("wkT", [DIM, DIM], f32r, False)
    wqT_d = nc.declare_dram_parameter("wqT", [DIM, DIM], f32r, False)
    wvT_d = nc.declare_dram_parameter("wvT", [DIM, DIM], f32r, False)
    bk_d = nc.declare_dram_parameter("bk", [P, MB], f32, False)
    bq_d = nc.declare_dram_parameter("bq", [P, MB], f32, False)
    bv_d = nc.declare_dram_parameter("bv", [1, DIM], f32r, False)
    madd_d = nc.declare_dram_parameter("madd", [P, nb], f32, False)
    gamma_d = nc.declare_dram_parameter("gamma", [P, MB], f32, False)
    beta_d = nc.declare_dram_parameter("beta", [P, MB], f32, False)
    sel_d = nc.declare_dram_parameter("sel", [4, 2], f32, False)
    onesr_d = nc.declare_dram_parameter("onesr", [1, P], f32r, False)
    out_d = nc.declare_dram_parameter("out", [P, MB], f32, True)

    with tile.TileContext(nc) as tc:
        with (
            tc.tile_pool(name="const", bufs=1) as const,
            tc.tile_pool(name="big", bufs=1) as big,
            tc.tile_pool(name="work", bufs=2) as work,
            tc.tile_pool(name="mmps", bufs=2, space="PSUM") as mm_psum,
        ):
            vec_psum = mm_psum  # tail psum tiles share the mm slots
            # ---- load constants/weights ----
            wk = [const.tile([P, DIM], f32r, tag=f"wk{k}", name=f"wk{k}") for k in range(MB)]
            wq = [const.tile([P, DIM], f32r, tag=f"wq{k}", name=f"wq{k}") for k in range(MB)]
            wv = [const.tile([P, DIM], f32r, tag=f"wv{k}", name=f"wv{k}") for k in range(MB)]
            bk_sb = const.tile([P, MB], f32, tag="bk")
            bq_sb = const.tile([P, MB], f32, tag="bq")
            bv_sb = const.tile([1, DIM], f32r, tag="bv")
            madd_sb = const.tile([P, nb], f32, tag="madd")
            gamma_sb = const.tile([P, MB], f32, tag="gamma")
            beta_sb = const.tile([P, MB], f32, tag="beta")
            nc.sync.dma_start(out=bk_sb, in_=bk_d[:, :])
            nc.sync.dma_start(out=bq_sb, in_=bq_d[:, :])
            nc.sync.dma_start(out=bv_sb, in_=bv_d[:, :])
            nc.sync.dma_start(out=madd_sb, in_=madd_d[:, :])
            nc.sync.dma_start(out=gamma_sb, in_=gamma_d[:, :])
            nc.sync.dma_start(out=beta_sb, in_=beta_d[:, :])
            ones_row = const.tile([1, P], f32, tag="ones")
            nc.gpsimd.memset(ones_row, 1.0)
            ones_row_r = const.tile([1, P], f32r, tag="onesr")
            nc.sync.dma_start(out=ones_row_r, in_=onesr_d[:, :])
            ones_col = const.tile([P, 1], f32, tag="onesc")
            nc.gpsimd.memset(ones_col, 1.0)
            eps_sb = const.tile([P, 1], f32, tag="eps")
            nc.gpsimd.memset(eps_sb, EPS)
            negs_sb = const.tile([P, 1], f32, tag="negs")
            nc.gpsimd.memset(negs_sb, -float(S))
            sel_sb = const.tile([4, 2], f32, tag="sel")
            nc.sync.dma_start(out=sel_sb, in_=sel_d[:, :])

            for rep in range(reps):
                xa = [big.tile([P, S], f32r, tag=f"xa{k}", name=f"xa{k}") for k in range(MB)]
                xb = [big.tile([P, S], f32r, tag=f"xb{k}", name=f"xb{k}") for k in range(MB)]
                # issue order tracks the compute critical path:
                # weights, k/v row slots of xa, q columns of xb, mean tail
                for k in range(MB):
                    nc.sync.dma_start(out=wk[k], in_=wkT_d[k * P:(k + 1) * P, :])
                for k in range(MB):
                    nc.sync.dma_start(out=wq[k], in_=wqT_d[k * P:(k + 1) * P, :])
                for k in range(MB):
                    nc.sync.dma_start(out=wv[k], in_=wvT_d[k * P:(k + 1) * P, :])
                for c0 in range(0, C, 1024):
                    w = min(1024, C - c0)
                    for k in range(MB):
                        nc.sync.dma_start(
                            out=xa[k][:, c0:c0 + w],
                            in_=xaT_d[k * P:(k + 1) * P, c0:c0 + w])
                for c0 in range(0, S, 1024):
                    for k in range(MB):
                        nc.sync.dma_start(
                            out=xb[k][:, c0:c0 + 1024],
                            in_=xbT_d[k * P:(k + 1) * P, c0:c0 + 1024])
                for c0 in range(C, S, 1024):
                    w = min(1024, S - c0)
                    for k in range(MB):
                        nc.sync.dma_start(
                            out=xa[k][:, c0:c0 + w],
                            in_=xaT_d[k * P:(k + 1) * P, c0:c0 + w])

                # ---- projections k (C slots), q (full S) in [d, s] layout ----
                # relu+bias on DVE so ACT stays dedicated to tanh/exp
                kt = [big.tile([P, C], f32r, tag=f"kt{k}", name=f"kt{k}") for k in range(MB)]
                qt = [big.tile([P, S], f32r, tag=f"qt{k}", name=f"qt{k}") for k in range(MB)]
                kchunks = [(c0, min(2048, C - c0)) for c0 in range(0, C, 2048)]
                for dst, wsb, bsb, src, chunks in (
                        (kt, wk, bk_sb, xa, kchunks),
                        (qt, wq, bq_sb, xb, [(0, 2048), (2048, 2048)])):
                    for m in range(MB):
                        for (c0, w) in chunks:
                            ps = mm_psum.tile([P, w], f32, tag="mm",
                                              name=f"kqps{c0}")
                            for kk in range(MB):
                                for h0 in range(0, w, JC):
                                    hw = min(JC, w - h0)
                                    nc.tensor.matmul(
                                        ps[:, h0:h0 + hw],
                                        lhsT=wsb[kk][:, m * P:(m + 1) * P],
                                        rhs=src[kk][:, c0 + h0:c0 + h0 + hw],
                                        start=(kk == 0), stop=(kk == MB - 1),
                                    )
                            nc.vector.tensor_scalar(
                                out=dst[m][:, c0:c0 + w], in0=ps,
                                scalar1=bsb[:, m:m + 1], scalar2=0.0,
                                op0=OP.add, op1=OP.max)

                # ---- scores + streaming softmax numerator ----
                # s[i] = sum_j tanh(k[i].q[j]); since s <= S the constant
                # shift -S is a safe softmax stabilizer, so each i-block's
                # exp(s - S) and its e*v contribution stream during the
                # score phase (no global max pass, no tail vec matmuls).
                # v-projection (natural [s, d] layout, 2 blocks per psum
                # tile) is interleaved one pair per i-block so the PE never
                # starves the ACT tanh pipeline.
                v_sb = big.tile([P, nb * DIM], f32, tag="v")
                s_sb = work.tile([P, nb], f32, tag="s")
                e_sb = work.tile([P, nb], f32, tag="e")
                cs_all = work.tile([P, nb, MB], f32, tag="cs")
                zcols = work.tile([P, ngroups], f32, tag="zc")
                vacc = work.tile([P, DIM], f32, tag="vacc")
                nc.vector.memset(vacc, 0.0)
                xsum = work.tile([P, MB], f32, tag="xsum")

                def emit_vpair(pi):
                    ps = mm_psum.tile([P, 2 * DIM], f32, tag="mm", name="vps")
                    for sub in range(2):
                        si = 2 * pi + sub
                        if si >= nb:
                            continue
                        for kk in range(MB):
                            nc.tensor.matmul(
                                ps[:, sub * DIM:(sub + 1) * DIM],
                                lhsT=xa[kk][:, si * P:(si + 1) * P],
                                rhs=wv[kk],
                                start=(kk == 0), stop=False,
                            )
                        nc.tensor.matmul(
                            ps[:, sub * DIM:(sub + 1) * DIM],
                            lhsT=ones_row_r, rhs=bv_sb,
                            start=False, stop=True,
                        )
                    w = min(2 * DIM, (nb - 2 * pi) * DIM)
                    nc.vector.tensor_scalar_max(
                        out=v_sb[:, 2 * pi * DIM:2 * pi * DIM + w],
                        in0=ps[:, :w], scalar1=0.0)

                npairs = (nb + 1) // 2
                for ib in range(nb):
                    for jg in range(2):  # [128,2048] psum tiles (4 banks)
                        ps = mm_psum.tile([P, 4 * JC], f32, tag="mm",
                                          name=f"scps{jg}")
                        for kk in range(MB):
                            for h in range(4):
                                n = jg * 4 + h
                                nc.tensor.matmul(
                                    ps[:, h * JC:(h + 1) * JC],
                                    lhsT=kt[kk][:, ib * P:(ib + 1) * P],
                                    rhs=qt[kk][:, n * JC:(n + 1) * JC],
                                    start=(kk == 0), stop=(kk == MB - 1),
                                )
                        # tanh in place in PSUM; row-sum via accum_out
                        nc.scalar.activation(
                            out=ps, in_=ps, func=AF.Tanh,
                            accum_out=cs_all[:, ib, jg:jg + 1],
                        )
                    if ib < npairs:
                        emit_vpair(ib)
                    if ib in ends:
                        g, lo, hi = ends[ib]
                        # masked score columns for the whole group, then
                        # exp and the streamed e*v accumulation
                        nc.vector.reduce_sum(
                            out=s_sb[:, lo:hi], in_=cs_all[:, lo:hi, :],
                            axis=AX.X)
                        nc.vector.tensor_add(
                            s_sb[:, lo:hi], s_sb[:, lo:hi],
                            madd_sb[:, lo:hi])
                        nc.scalar.activation(
                            out=e_sb[:, lo:hi],
                            in_=s_sb[:, lo:hi],
                            func=AF.Exp, bias=negs_sb,
                            accum_out=zcols[:, g:g + 1])
                        for b8 in range(lo, hi):
                            # vacc += e[block] * v[block]
                            nc.vector.scalar_tensor_tensor(
                                out=vacc,
                                in0=v_sb[:, b8 * DIM:(b8 + 1) * DIM],
                                scalar=e_sb[:, b8:b8 + 1], in1=vacc,
                                op0=OP.mult, op1=OP.add)
                        if g == 0:
                            # mean over seq: row-sum of xa, scheduled in
                            # the DVE slack inside the score phase
                            for k in range(MB):
                                nc.vector.reduce_sum(
                                    out=xsum[:, k:k + 1], in_=xa[k],
                                    axis=AX.X)

                # ---- softmax denominator ----
                zp = work.tile([P, 1], f32, tag="zp")
                nc.vector.reduce_sum(out=zp, in_=zcols, axis=AX.X)
                z_ps = vec_psum.tile([1, 1], f32, tag="mm", name="z_ps")
                nc.tensor.matmul(z_ps, lhsT=zp,
                                 rhs=ones_col, start=True, stop=True)
                z_sb = work.tile([1, 1], f32, tag="z1")
                nc.vector.tensor_copy(out=z_sb, in_=z_ps)
                invz1 = work.tile([1, 1], f32, tag="invz1")
                nc.vector.reciprocal(out=invz1, in_=z_sb)
                invz_ps = vec_psum.tile([P, 1], f32, tag="mm", name="invz_ps")
                nc.tensor.matmul(invz_ps, lhsT=ones_row,
                                 rhs=invz1, start=True, stop=True)
                invz = work.tile([P, 1], f32, tag="invz")
                nc.vector.tensor_copy(out=invz, in_=invz_ps)

                # ---- vec: partition-sum the streamed accumulator ----
                vecp = vec_psum.tile([P, MB], f32, tag="mm", name="vecp")
                for m in range(MB):
                    nc.tensor.matmul(
                        vecp[:, m:m + 1],
                        lhsT=vacc[:, m * P:(m + 1) * P],
                        rhs=ones_col, start=True, stop=True)

                # ---- y = mean(xa) + vec/Z ----
                vscaled = work.tile([P, MB], f32, tag="vs")
                nc.vector.tensor_scalar_mul(out=vscaled, in0=vecp, scalar1=invz)
                # stat4 cols: [y0, y1, y0^2, y1^2]; y_sb aliases cols 0:2
                stat4 = work.tile([P, 4], f32, tag="stat4")
                y_sb = stat4[:, 0:MB]
                nc.vector.scalar_tensor_tensor(
                    out=y_sb, in0=xsum, scalar=1.0 / S, in1=vscaled,
                    op0=OP.mult, op1=OP.add)

                # ---- layernorm over d=256 (spans 2 partition blocks) ----
                nc.vector.tensor_mul(stat4[:, MB:2 * MB], y_sb, y_sb)
                r4_ps = vec_psum.tile([4, 1], f32, tag="mm", name="r4_ps")
                nc.tensor.matmul(r4_ps, lhsT=stat4,
                                 rhs=ones_col, start=True, stop=True)
                r4 = work.tile([4, 1], f32, tag="r4")
                nc.vector.tensor_copy(out=r4, in_=r4_ps)
                s12_ps = vec_psum.tile([1, 2], f32, tag="mm", name="s12_ps")
                nc.tensor.matmul(s12_ps, lhsT=r4,
                                 rhs=sel_sb, start=True, stop=True)
                s12 = work.tile([1, 2], f32, tag="s12")
                nc.vector.tensor_copy(out=s12, in_=s12_ps)
                # mu = sum(y)/D ; ex2 = sum(y^2)/D ; var = ex2 - mu^2
                ms = work.tile([1, 2], f32, tag="ms")
                nc.vector.tensor_scalar_mul(out=ms, in0=s12,
                                            scalar1=1.0 / DIM)
                mu2 = work.tile([1, 1], f32, tag="mu2")
                nc.vector.tensor_mul(mu2, ms[:, 0:1], ms[:, 0:1])
                var = work.tile([1, 1], f32, tag="var")
                nc.vector.tensor_sub(var, ms[:, 1:2], mu2)
                # rstd = exp(-0.5*ln(var+eps))  (ln/exp share a table set)
                lnv = work.tile([1, 1], f32, tag="lnv")
                nc.scalar.activation(out=lnv, in_=var, func=AF.Ln,
                                     bias=eps_sb[0:1, :])
                mr1 = work.tile([1, 2], f32, tag="mr1")
                nc.vector.tensor_copy(out=mr1[:, 0:1], in_=ms[:, 0:1])
                nc.scalar.activation(out=mr1[:, 1:2], in_=lnv, func=AF.Exp,
                                     scale=-0.5)
                # broadcast [mu, rstd] to all partitions
                mr_ps = vec_psum.tile([P, 2], f32, tag="mm", name="mr_ps")
                nc.tensor.matmul(mr_ps, lhsT=ones_row,
                                 rhs=mr1, start=True, stop=True)
                mr_sb = work.tile([P, 2], f32, tag="mr")
                nc.vector.tensor_copy(out=mr_sb, in_=mr_ps)
                # (y - mu) * rstd
                norm = work.tile([P, MB], f32, tag="norm")
                nc.vector.tensor_scalar(
                    out=norm, in0=y_sb, scalar1=mr_sb[:, 0:1],
                    scalar2=mr_sb[:, 1:2], op0=OP.subtract, op1=OP.mult)
                normg = work.tile([P, MB], f32, tag="normg")
                nc.vector.tensor_mul(normg, norm, gamma_sb)
                out_sb = work.tile([P, MB], f32, tag="out")
                nc.vector.tensor_add(out_sb, normg, beta_sb)
                nc.sync.dma_start(out=out_d[:, :], in_=out_sb)

    nc.finalize()
    return nc


def _get_program(reps=1, nb=NB):
    key = (reps, nb)
    if key not in _PROG:
        _PROG[key] = _build_program(reps, nb)
    return _PROG[key]


def _pn(v):
    """[DIM] -> [P, MB] with tile[p, m] = v[m*128 + p]."""
    return np.ascontiguousarray(np.asarray(v, np.float32).reshape(MB, P).T)


def make_in_maps(fingerprint_vectors1, fingerprint_vectors2, mask1, mask2,
                 Wq, bq, Wk, bk, Wv, bv, gamma, beta, nb=NB):
    x1 = np.asarray(fingerprint_vectors1, np.float32)
    x2 = np.asarray(fingerprint_vectors2, np.float32)
    m1 = np.asarray(mask1, bool)
    m2 = np.asarray(mask2, bool)
    x1T = np.ascontiguousarray(x1.transpose(0, 2, 1))  # [B, D, S]
    x2T = np.ascontiguousarray(x2.transpose(0, 2, 1))
    wqT = np.ascontiguousarray(np.asarray(Wq, np.float32).T)
    wkT = np.ascontiguousarray(np.asarray(Wk, np.float32).T)
    wvT = np.ascontiguousarray(np.asarray(Wv, np.float32).T)
    shared = {
        "wkT": wkT, "wqT": wqT, "wvT": wvT,
        "bk": _pn(bk), "bq": _pn(bq),
        "bv": np.ascontiguousarray(np.asarray(bv, np.float32).reshape(1, DIM)),
        "gamma": _pn(gamma), "beta": _pn(beta),
        "sel": np.array([[1, 0], [1, 0], [0, 1], [0, 1]], np.float32),
        "onesr": np.ones((1, P), np.float32),
    }
    in_maps = []
    slots = nb * P
    for b in range(B):
        for stream in range(2):
            if stream == 0:
                xs, xbt, msk = x1[b], x2T[b], m1[b]
            else:
                xs, xbt, msk = x2[b], x1T[b], m2[b]
            # own-stream rows permuted to [unmasked | masked]; only the
            # first `slots` row slots are computed on device, the rest
            # have softmax weight exactly 0
            perm = np.argsort(msk, kind="stable")
            xaP = np.ascontiguousarray(xs[perm].T)
            n_um = int((~msk).sum())
            madd = np.full(slots, np.float32(-1e30), np.float32)
            madd[:min(n_um, slots)] = 0.0
            madd = np.ascontiguousarray(madd.reshape(nb, P).T)
            in_maps.append(dict(shared, xaT=xaP, xbT=xbt, madd=madd))
    return in_maps


# test.py can flip these to get a profile out of the run
RUN_OPTS = {"trace": False, "trace_kwargs": None}
LAST = {}


def kernel(**inputs):
    from concourse.bass_utils import run_bass_kernel_spmd

    m1 = np.asarray(inputs["mask1"], bool)
    m2 = np.asarray(inputs["mask2"], bool)
    n_um_max = max(int((~m1).sum(axis=1).max()), int((~m2).sum(axis=1).max()))
    nb = NB if n_um_max <= NB * P else S // P

    nc = _get_program(1, nb)
    in_maps = make_in_maps(nb=nb, **inputs)
    kw = {}
    if RUN_OPTS.get("trace"):
        kw["trace"] = True
        if RUN_OPTS.get("trace_kwargs"):
            kw["trace_kwargs"] = RUN_OPTS["trace_kwargs"]
    res = run_bass_kernel_spmd(nc, in_maps, list(range(NCORES)), **kw)
    LAST["exec_time_ns"] = res.exec_time_ns
    LAST["profile_json"] = res.profile_json
    outs = res.results
    out1 = np.stack([np.asarray(outs[2 * b]["out"]).T.reshape(DIM)
                     for b in range(B)])
    out2 = np.stack([np.asarray(outs[2 * b + 1]["out"]).T.reshape(DIM)
                     for b in range(B)])
    return out1.astype(np.float32), out2.astype(np.float32)


# revision 10
# speedup vs baseline: 2.6136x; 1.9089x over previous
"""Dual-stream attention kernel for Trainium2 (8 NeuronCores, SPMD).

Problem: B=4, S=4096, DIM=256
  out1 = LN(mean(x1,1) + softmax(mask(sum_j tanh(k1 @ q2.T))) @ v1)
  out2 = LN(mean(x2,1) + softmax(mask(sum_j tanh(k2 @ q1.T))) @ v2)

Sharding: 8 independent (batch, stream) units -> one per core, no
cross-core communication. Core 2*b+s handles batch b, stream s.

Mask trimming: softmax rows i with mask=True get weight exactly 0, so
k/v/scores are only needed for unmasked rows. The host permutes each
core's own-stream x into [unmasked | masked] column order and the
device computes only the first NB*128 row slots (capacity ~2048+8
sigma for a random half mask); slots past the unmasked count are
killed by the -1e30 mask-add exactly like the full kernel would.
A full 32-block program is kept as a fallback if a mask ever exceeds
capacity.
"""

import numpy as np

B, S, DIM = 4, 4096, 256
P = 128
MB = DIM // P      # 2 d-blocks of 128 partitions
NB = 17            # row-slot capacity in blocks of 128 (C = 2176)
JC = 512           # score j-chunk (one PSUM bank of fp32)
EPS = 1e-5
NCORES = 8

_PROG = {}         # cached Bass programs by (reps, nb)


def _group_sizes(nb):
    # exp/e*v groups over the i-blocks; shrinking tail groups shorten
    # the final accumulation chain
    if nb == 17:
        return [6, 6, 3, 2]
    if nb == 18:
        return [6, 6, 3, 2, 1]
    if nb == 32:
        return [8, 8, 8, 4, 2, 2]
    sizes, rem = [], nb
    while rem > 0:
        g = max(1, min(rem // 2, 8)) if rem > 2 else rem
        sizes.append(g)
        rem -= g
    return sizes


def _build_program(reps=1, nb=NB):
    import concourse.bacc as bacc
    import concourse.tile as tile
    from concourse import mybir

    f32 = mybir.dt.float32
    f32r = mybir.dt.float32r
    f8 = mybir.dt.float8e4
    AF = mybir.ActivationFunctionType
    AX = mybir.AxisListType
    OP = mybir.AluOpType

    C = nb * P
    sizes = _group_sizes(nb)
    ends = {}
    lo = 0
    for g, sz in enumerate(sizes):
        ends[lo + sz - 1] = (g, lo, lo + sz)
        lo += sz
    ngroups = len(sizes)

    nc = bacc.Bacc("TRN2", target_bir_lowering=False, debug=False)

    # ---- DRAM I/O (per-core data; weights replicated) ----
    xaT_d = nc.declare_dram_parameter("xaT", [DIM, S], f32r, False)
    xbT_d = nc.declare_dram_parameter("xbT", [DIM, S], f32r, False)
    wkT_d = nc.declare_dram_parameter("wkT", [DIM, DIM], f32r, False)
    wqT_d = nc.declare_dram_parameter("wqT", [DIM, DIM], f32r, False)
    wvT_d = nc.declare_dram_parameter("wvT", [DIM, DIM], f32r, False)
    bk_d = nc.declare_dram_parameter("bk", [P, MB], f32, False)
    bq_d = nc.declare_dram_parameter("bq", [P, MB], f32, False)
    bv_d = nc.declare_dram_parameter("bv", [1, DIM], f32r, False)
    madd_d = nc.declare_dram_parameter("madd", [P, nb], f32, False)
    gamma_d = nc.declare_dram_parameter("gamma", [P, MB], f32, False)
    beta_d = nc.declare_dram_parameter("beta", [P, MB], f32, False)
    sel_d = nc.declare_dram_parameter("sel", [4, 2], f32, False)
    onesr_d = nc.declare_dram_parameter("onesr", [1, P], f32r, False)
    out_d = nc.declare_dram_parameter("out", [P, MB], f32, True)

    with tile.TileContext(nc) as tc:
        with (
            tc.tile_pool(name="const", bufs=1) as const,
            tc.tile_pool(name="big", bufs=1) as big,
            tc.tile_pool(name="work", bufs=2) as work,
            tc.tile_pool(name="mmps", bufs=2, space="PSUM") as mm_psum,
        ):
            vec_psum = mm_psum  # tail psum tiles share the mm slots
            # ---- load constants/weights ----
            wk = [const.tile([P, DIM], f32r, tag=f"wk{k}", name=f"wk{k}") for k in range(MB)]
            wq = [const.tile([P, DIM], f32r, tag=f"wq{k}", name=f"wq{k}") for k in range(MB)]
            wv = [const.tile([P, DIM], f32r, tag=f"wv{k}", name=f"wv{k}") for k in range(MB)]
            bk_sb = const.tile([P, MB], f32, tag="bk")
            bq_sb = const.tile([P, MB], f32, tag="bq")
            bv_sb = const.tile([1, DIM], f32r, tag="bv")
            madd_sb = const.tile([P, nb], f32, tag="madd")
            gamma_sb = const.tile([P, MB], f32, tag="gamma")
            beta_sb = const.tile([P, MB], f32, tag="beta")
            nc.sync.dma_start(out=bk_sb, in_=bk_d[:, :])
            nc.sync.dma_start(out=bq_sb, in_=bq_d[:, :])
            nc.sync.dma_start(out=bv_sb, in_=bv_d[:, :])
            nc.sync.dma_start(out=madd_sb, in_=madd_d[:, :])
            nc.sync.dma_start(out=gamma_sb, in_=gamma_d[:, :])
            nc.sync.dma_start(out=beta_sb, in_=beta_d[:, :])
            ones_row = const.tile([1, P], f32, tag="ones")
            nc.gpsimd.memset(ones_row, 1.0)
            ones_row_r = const.tile([1, P], f32r, tag="onesr")
            nc.sync.dma_start(out=ones_row_r, in_=onesr_d[:, :])
            ones_col = const.tile([P, 1], f32, tag="onesc")
            nc.gpsimd.memset(ones_col, 1.0)
            eps_sb = const.tile([P, 1], f32, tag="eps")
            nc.gpsimd.memset(eps_sb, EPS)
            negs_sb = const.tile([P, 1], f32, tag="negs")
            nc.gpsimd.memset(negs_sb, -float(S))
            sel_sb = const.tile([4, 2], f32, tag="sel")
            nc.sync.dma_start(out=sel_sb, in_=sel_d[:, :])
            for k in range(MB):
                nc.sync.dma_start(out=wk[k], in_=wkT_d[k * P:(k + 1) * P, :])
            for k in range(MB):
                nc.sync.dma_start(out=wq[k], in_=wqT_d[k * P:(k + 1) * P, :])
            for k in range(MB):
                nc.sync.dma_start(out=wv[k], in_=wvT_d[k * P:(k + 1) * P, :])

            for rep in range(reps):
                xa = [big.tile([P, S], f32r, tag=f"xa{k}", name=f"xa{k}") for k in range(MB)]
                xb = [big.tile([P, S], f32r, tag=f"xb{k}", name=f"xb{k}") for k in range(MB)]
                # issue order tracks the compute critical path:
                # k/v row slots of xa, q columns of xb, mean tail
                for c0 in range(0, C, 1024):
                    w = min(1024, C - c0)
                    for k in range(MB):
                        nc.sync.dma_start(
                            out=xa[k][:, c0:c0 + w],
                            in_=xaT_d[k * P:(k + 1) * P, c0:c0 + w])
                for c0 in range(0, S, 1024):
                    for k in range(MB):
                        nc.sync.dma_start(
                            out=xb[k][:, c0:c0 + 1024],
                            in_=xbT_d[k * P:(k + 1) * P, c0:c0 + 1024])
                for c0 in range(C, S, 1024):
                    w = min(1024, S - c0)
                    for k in range(MB):
                        nc.sync.dma_start(
                            out=xa[k][:, c0:c0 + w],
                            in_=xaT_d[k * P:(k + 1) * P, c0:c0 + w])

                # ---- projections k (C slots), q (full S) in [d, s] layout ----
                # relu+bias on DVE so ACT stays dedicated to tanh/exp.
                # k/q are stored as fp8e4 [P, 2, cols] (the two d-halves
                # stacked) so the score matmul runs in DoubleRow perf mode:
                # the full 256-deep contraction in one 0.5-cycle/row op.
                # Post-relu values are O(1) so e4m3 (max 240, rel step
                # 2^-4) quantization shifts the dots by well under 1%,
                # and the tanh saturation margin (all dots >= ~15 vs
                # tanh==1.0f cutoff at 9.01) makes that irrelevant.
                # bufs=2 so the next rep's projections overlap this rep's
                # score tail.
                kt8 = big.tile([P, 2, C], f8, tag="kt8", bufs=2)
                qt8 = big.tile([P, 2, S], f8, tag="qt8", bufs=2)
                kchunks = [(c0, min(2048, C - c0)) for c0 in range(0, C, 2048)]
                for dst, wsb, bsb, src, chunks in (
                        (kt8, wk, bk_sb, xa, kchunks),
                        (qt8, wq, bq_sb, xb, [(0, 2048), (2048, 2048)])):
                    for m in range(MB):
                        for (c0, w) in chunks:
                            ps = mm_psum.tile([P, w], f32, tag="mm",
                                              name=f"kqps{c0}")
                            for kk in range(MB):
                                for h0 in range(0, w, JC):
                                    hw = min(JC, w - h0)
                                    nc.tensor.matmul(
                                        ps[:, h0:h0 + hw],
                                        lhsT=wsb[kk][:, m * P:(m + 1) * P],
                                        rhs=src[kk][:, c0 + h0:c0 + h0 + hw],
                                        start=(kk == 0), stop=(kk == MB - 1),
                                    )
                            nc.vector.tensor_scalar(
                                out=dst[:, m, c0:c0 + w], in0=ps,
                                scalar1=bsb[:, m:m + 1], scalar2=0.0,
                                op0=OP.add, op1=OP.max)

                # ---- scores + streaming softmax numerator ----
                # s[i] = sum_j tanh(k[i].q[j]); since s <= S the constant
                # shift -S is a safe softmax stabilizer, so each i-block's
                # exp(s - S) and its e*v contribution stream during the
                # score phase (no global max pass, no tail vec matmuls).
                # v-projection (natural [s, d] layout, 2 blocks per psum
                # tile) is interleaved one pair per i-block so the PE never
                # starves the ACT tanh pipeline.
                v_sb = big.tile([P, nb * DIM], f32, tag="v")
                s_sb = work.tile([P, nb], f32, tag="s")
                e_sb = work.tile([P, nb], f32, tag="e")
                cs_all = work.tile([P, nb, MB], f32, tag="cs")
                zcols = work.tile([P, ngroups], f32, tag="zc")
                vacc = work.tile([P, DIM], f32, tag="vacc")
                nc.vector.memset(vacc, 0.0)
                xsum = work.tile([P, MB], f32, tag="xsum")

                def emit_vpair(pi):
                    ps = mm_psum.tile([P, 2 * DIM], f32, tag="mm", name="vps")
                    for sub in range(2):
                        si = 2 * pi + sub
                        if si >= nb:
                            continue
                        for kk in range(MB):
                            nc.tensor.matmul(
                                ps[:, sub * DIM:(sub + 1) * DIM],
                                lhsT=xa[kk][:, si * P:(si + 1) * P],
                                rhs=wv[kk],
                                start=(kk == 0), stop=False,
                            )
                        nc.tensor.matmul(
                            ps[:, sub * DIM:(sub + 1) * DIM],
                            lhsT=ones_row_r, rhs=bv_sb,
                            start=False, stop=True,
                        )
                    w = min(2 * DIM, (nb - 2 * pi) * DIM)
                    nc.vector.tensor_scalar_max(
                        out=v_sb[:, 2 * pi * DIM:2 * pi * DIM + w],
                        in0=ps[:, :w], scalar1=0.0)

                npairs = (nb + 1) // 2
                for ib in range(nb):
                    for jg in range(2):  # [128,2048] psum tiles (4 banks)
                        ps = mm_psum.tile([P, 4 * JC], f32, tag="mm",
                                          name=f"scps{jg}")
                        for h in range(4):
                            n = jg * 4 + h
                            nc.tensor.matmul(
                                ps[:, h * JC:(h + 1) * JC],
                                lhsT=kt8[:, :, ib * P:(ib + 1) * P],
                                rhs=qt8[:, :, n * JC:(n + 1) * JC],
                                start=True, stop=True,
                                perf_mode=mybir.MatmulPerfMode.DoubleRow,
                            )
                        # tanh in place in PSUM; row-sum via accum_out
                        nc.scalar.activation(
                            out=ps, in_=ps, func=AF.Tanh,
                            accum_out=cs_all[:, ib, jg:jg + 1],
                        )
                    for t in (2 * ib, 2 * ib + 1):
                        if t < npairs:
                            emit_vpair(t)
                    if ib in ends:
                        g, lo, hi = ends[ib]
                        # masked score columns for the whole group, then
                        # exp and the streamed e*v accumulation
                        nc.vector.reduce_sum(
                            out=s_sb[:, lo:hi], in_=cs_all[:, lo:hi, :],
                            axis=AX.X)
                        nc.vector.tensor_add(
                            s_sb[:, lo:hi], s_sb[:, lo:hi],
                            madd_sb[:, lo:hi])
                        nc.scalar.activation(
                            out=e_sb[:, lo:hi],
                            in_=s_sb[:, lo:hi],
                            func=AF.Exp, bias=negs_sb,
                            accum_out=zcols[:, g:g + 1])
                        for b8 in range(lo, hi):
                            # vacc += e[block] * v[block]
                            nc.vector.scalar_tensor_tensor(
                                out=vacc,
                                in0=v_sb[:, b8 * DIM:(b8 + 1) * DIM],
                                scalar=e_sb[:, b8:b8 + 1], in1=vacc,
                                op0=OP.mult, op1=OP.add)
                        if g == 0:
                            # mean over seq: row-sum of xa, scheduled in
                            # the DVE slack inside the score phase
                            for k in range(MB):
                                nc.vector.reduce_sum(
                                    out=xsum[:, k:k + 1], in_=xa[k],
                                    axis=AX.X)

                # ---- softmax denominator ----
                zp = work.tile([P, 1], f32, tag="zp")
                nc.vector.reduce_sum(out=zp, in_=zcols, axis=AX.X)
                z_ps = vec_psum.tile([1, 1], f32, tag="mm", name="z_ps")
                nc.tensor.matmul(z_ps, lhsT=zp,
                                 rhs=ones_col, start=True, stop=True)
                z_sb = work.tile([1, 1], f32, tag="z1")
                nc.vector.tensor_copy(out=z_sb, in_=z_ps)
                invz1 = work.tile([1, 1], f32, tag="invz1")
                nc.vector.reciprocal(out=invz1, in_=z_sb)
                invz_ps = vec_psum.tile([P, 1], f32, tag="mm", name="invz_ps")
                nc.tensor.matmul(invz_ps, lhsT=ones_row,
                                 rhs=invz1, start=True, stop=True)
                invz = work.tile([P, 1], f32, tag="invz")
                nc.vector.tensor_copy(out=invz, in_=invz_ps)

                # ---- vec: partition-sum the streamed accumulator ----
                vecp = vec_psum.tile([P, MB], f32, tag="mm", name="vecp")
                for m in range(MB):
                    nc.tensor.matmul(
                        vecp[:, m:m + 1],
                        lhsT=vacc[:, m * P:(m + 1) * P],
                        rhs=ones_col, start=True, stop=True)

                # ---- y = mean(xa) + vec/Z ----
                vscaled = work.tile([P, MB], f32, tag="vs")
                nc.vector.tensor_scalar_mul(out=vscaled, in0=vecp, scalar1=invz)
                # stat4 cols: [y0, y1, y0^2, y1^2]; y_sb aliases cols 0:2
                stat4 = work.tile([P, 4], f32, tag="stat4")
                y_sb = stat4[:, 0:MB]
                nc.vector.scalar_tensor_tensor(
                    out=y_sb, in0=xsum, scalar=1.0 / S, in1=vscaled,
                    op0=OP.mult, op1=OP.add)

                # ---- layernorm over d=256 (spans 2 partition blocks) ----
                nc.vector.tensor_mul(stat4[:, MB:2 * MB], y_sb, y_sb)
                r4_ps = vec_psum.tile([4, 1], f32, tag="mm", name="r4_ps")
                nc.tensor.matmul(r4_ps, lhsT=stat4,
                                 rhs=ones_col, start=True, stop=True)
                r4 = work.tile([4, 1], f32, tag="r4")
                nc.vector.tensor_copy(out=r4, in_=r4_ps)
                s12_ps = vec_psum.tile([1, 2], f32, tag="mm", name="s12_ps")
                nc.tensor.matmul(s12_ps, lhsT=r4,
                                 rhs=sel_sb, start=True, stop=True)
                s12 = work.tile([1, 2], f32, tag="s12")
                nc.vector.tensor_copy(out=s12, in_=s12_ps)
                # mu = sum(y)/D ; ex2 = sum(y^2)/D ; var = ex2 - mu^2
                ms = work.tile([1, 2], f32, tag="ms")
                nc.vector.tensor_scalar_mul(out=ms, in0=s12,
                                            scalar1=1.0 / DIM)
                mu2 = work.tile([1, 1], f32, tag="mu2")
                nc.vector.tensor_mul(mu2, ms[:, 0:1], ms[:, 0:1])
                var = work.tile([1, 1], f32, tag="var")
                nc.vector.tensor_sub(var, ms[:, 1:2], mu2)
                # rstd = exp(-0.5*ln(var+eps))  (ln/exp share a table set)
                lnv = work.tile([1, 1], f32, tag="lnv")
                nc.scalar.activation(out=lnv, in_=var, func=AF.Ln,
                                     bias=eps_sb[0:1, :])
                mr1 = work.tile([1, 2], f32, tag="mr1")
                nc.vector.tensor_copy(out=mr1[:, 0:1], in_=ms[:, 0:1])
                nc.scalar.activation(out=mr1[:, 1:2], in_=lnv, func=AF.Exp,
                                     scale=-0.5)
                # broadcast [mu, rstd] to all partitions
                mr_ps = vec_psum.tile([P, 2], f32, tag="mm", name="mr_ps")
                nc.tensor.matmul(mr_ps, lhsT=ones_row,
                                 rhs=mr1, start=True, stop=True)
                mr_sb = work.tile([P, 2], f32, tag="mr")
                nc.vector.tensor_copy(out=mr_sb, in_=mr_ps)
                # (y - mu) * rstd
                norm = work.tile([P, MB], f32, tag="norm")
                nc.vector.tensor_scalar(
                    out=norm, in0=y_sb, scalar1=mr_sb[:, 0:1],
                    scalar2=mr_sb[:, 1:2], op0=OP.subtract, op1=OP.mult)
                normg = work.tile([P, MB], f32, tag="normg")
                nc.vector.tensor_mul(normg, norm, gamma_sb)
                out_sb = work.tile([P, MB], f32, tag="out")
                nc.vector.tensor_add(out_sb, normg, beta_sb)
                nc.sync.dma_start(out=out_d[:, :], in_=out_sb)

    nc.finalize()
    return nc


def _get_program(reps=1, nb=NB):
    key = (reps, nb)
    if key not in _PROG:
        _PROG[key] = _build_program(reps, nb)
    return _PROG[key]


def _pn(v):
    """[DIM] -> [P, MB] with tile[p, m] = v[m*128 + p]."""
    return np.ascontiguousarray(np.asarray(v, np.float32).reshape(MB, P).T)


def make_in_maps(fingerprint_vectors1, fingerprint_vectors2, mask1, mask2,
                 Wq, bq, Wk, bk, Wv, bv, gamma, beta, nb=NB):
    x1 = np.asarray(fingerprint_vectors1, np.float32)
    x2 = np.asarray(fingerprint_vectors2, np.float32)
    m1 = np.asarray(mask1, bool)
    m2 = np.asarray(mask2, bool)
    x1T = np.ascontiguousarray(x1.transpose(0, 2, 1))  # [B, D, S]
    x2T = np.ascontiguousarray(x2.transpose(0, 2, 1))
    wqT = np.ascontiguousarray(np.asarray(Wq, np.float32).T)
    wkT = np.ascontiguousarray(np.asarray(Wk, np.float32).T)
    wvT = np.ascontiguousarray(np.asarray(Wv, np.float32).T)
    shared = {
        "wkT": wkT, "wqT": wqT, "wvT": wvT,
        "bk": _pn(bk), "bq": _pn(bq),
        "bv": np.ascontiguousarray(np.asarray(bv, np.float32).reshape(1, DIM)),
        "gamma": _pn(gamma), "beta": _pn(beta),
        "sel": np.array([[1, 0], [1, 0], [0, 1], [0, 1]], np.float32),
        "onesr": np.ones((1, P), np.float32),
    }
    in_maps = []
    slots = nb * P
    for b in range(B):
        for stream in range(2):
            if stream == 0:
                xs, xbt, msk = x1[b], x2T[b], m1[b]
            else:
                xs, xbt, msk = x2[b], x1T[b], m2[b]
            # own-stream rows permuted to [unmasked | masked]; only the
            # first `slots` row slots are computed on device, the rest
            # have softmax weight exactly 0
            perm = np.argsort(msk, kind="stable")
            xaP = np.ascontiguousarray(xs[perm].T)
            n_um = int((~msk).sum())
            madd = np.full(slots, np.float32(-1e30), np.float32)
            madd[:min(n_um, slots)] = 0.0
            madd = np.ascontiguousarray(madd.reshape(nb, P).T)
            in_maps.append(dict(shared, xaT=xaP, xbT=xbt, madd=madd))
    return in_maps


# test.py can flip these to get a profile out of the run
RUN_OPTS = {"trace": False, "trace_kwargs": None}
LAST = {}


def kernel(**inputs):
    from concourse.bass_utils import run_bass_kernel_spmd

    m1 = np.asarray(inputs["mask1"], bool)
    m2 = np.asarray(inputs["mask2"], bool)
    n_um_max = max(int((~m1).sum(axis=1).max()), int((~m2).sum(axis=1).max()))
    nb = NB if n_um_max <= NB * P else S // P

    nc = _get_program(1, nb)
    in_maps = make_in_maps(nb=nb, **inputs)
    kw = {}
    if RUN_OPTS.get("trace"):
        kw["trace"] = True
        if RUN_OPTS.get("trace_kwargs"):
            kw["trace_kwargs"] = RUN_OPTS["trace_kwargs"]
    res = run_bass_kernel_spmd(nc, in_maps, list(range(NCORES)), **kw)
    LAST["exec_time_ns"] = res.exec_time_ns
    LAST["profile_json"] = res.profile_json
    outs = res.results
    out1 = np.stack([np.asarray(outs[2 * b]["out"]).T.reshape(DIM)
                     for b in range(B)])
    out2 = np.stack([np.asarray(outs[2 * b + 1]["out"]).T.reshape(DIM)
                     for b in range(B)])
    return out1.astype(np.float32), out2.astype(np.float32)


# revision 13
# speedup vs baseline: 2.6597x; 1.0177x over previous
"""Dual-stream attention kernel for Trainium2 (8 NeuronCores, SPMD).

Problem: B=4, S=4096, DIM=256
  out1 = LN(mean(x1,1) + softmax(mask(sum_j tanh(k1 @ q2.T))) @ v1)
  out2 = LN(mean(x2,1) + softmax(mask(sum_j tanh(k2 @ q1.T))) @ v2)

Sharding: 8 independent (batch, stream) units -> one per core, no
cross-core communication. Core 2*b+s handles batch b, stream s.

Mask trimming: softmax rows i with mask=True get weight exactly 0, so
k/v/scores are only needed for unmasked rows. The host permutes each
core's own-stream x into [unmasked | masked] column order and the
device computes only the first NB*128 row slots (capacity ~2048+8
sigma for a random half mask); slots past the unmasked count are
killed by the -1e30 mask-add exactly like the full kernel would.
A full 32-block program is kept as a fallback if a mask ever exceeds
capacity.
"""

import numpy as np

B, S, DIM = 4, 4096, 256
P = 128
MB = DIM // P      # 2 d-blocks of 128 partitions
NB = 17            # row-slot capacity in blocks of 128 (C = 2176)
JC = 512           # score j-chunk (one PSUM bank of fp32)
EPS = 1e-5
NCORES = 8

_PROG = {}         # cached Bass programs by (reps, nb)


def _group_sizes(nb):
    # exp/e*v groups over the i-blocks; shrinking tail groups shorten
    # the final accumulation chain
    if nb == 17:
        return [6, 6, 3, 2]
    if nb == 18:
        return [6, 6, 3, 2, 1]
    if nb == 32:
        return [8, 8, 8, 4, 2, 2]
    sizes, rem = [], nb
    while rem > 0:
        g = max(1, min(rem // 2, 8)) if rem > 2 else rem
        sizes.append(g)
        rem -= g
    return sizes


def _build_program(reps=1, nb=NB):
    import concourse.bacc as bacc
    import concourse.tile as tile
    from concourse import mybir

    f32 = mybir.dt.float32
    f32r = mybir.dt.float32r
    f8 = mybir.dt.float8e4
    AF = mybir.ActivationFunctionType
    AX = mybir.AxisListType
    OP = mybir.AluOpType

    C = nb * P
    sizes = _group_sizes(nb)
    ends = {}
    lo = 0
    for g, sz in enumerate(sizes):
        ends[lo + sz - 1] = (g, lo, lo + sz)
        lo += sz
    ngroups = len(sizes)

    nc = bacc.Bacc("TRN2", target_bir_lowering=False, debug=False)

    # ---- DRAM I/O (per-core data; weights replicated) ----
    xaT_d = nc.declare_dram_parameter("xaT", [DIM, S], f32r, False)
    xbT_d = nc.declare_dram_parameter("xbT", [DIM, S], f32r, False)
    wkT_d = nc.declare_dram_parameter("wkT", [DIM, DIM], f32r, False)
    wqT_d = nc.declare_dram_parameter("wqT", [DIM, DIM], f32r, False)
    wvT_d = nc.declare_dram_parameter("wvT", [DIM, DIM], f32r, False)
    bk_d = nc.declare_dram_parameter("bk", [P, MB], f32, False)
    bq_d = nc.declare_dram_parameter("bq", [P, MB], f32, False)
    bv_d = nc.declare_dram_parameter("bv", [1, DIM], f32r, False)
    madd_d = nc.declare_dram_parameter("madd", [P, nb], f32, False)
    gamma_d = nc.declare_dram_parameter("gamma", [P, MB], f32, False)
    beta_d = nc.declare_dram_parameter("beta", [P, MB], f32, False)
    sel_d = nc.declare_dram_parameter("sel", [4, 2], f32, False)
    onesr_d = nc.declare_dram_parameter("onesr", [1, P], f32r, False)
    out_d = nc.declare_dram_parameter("out", [P, MB], f32, True)

    with tile.TileContext(nc) as tc:
        with (
            tc.tile_pool(name="const", bufs=1) as const,
            tc.tile_pool(name="big", bufs=1) as big,
            tc.tile_pool(name="work", bufs=2) as work,
            tc.tile_pool(name="mmps", bufs=2, space="PSUM") as mm_psum,
        ):
            vec_psum = mm_psum  # tail psum tiles share the mm slots
            # ---- load constants/weights ----
            wk = [const.tile([P, DIM], f32r, tag=f"wk{k}", name=f"wk{k}") for k in range(MB)]
            wq = [const.tile([P, DIM], f32r, tag=f"wq{k}", name=f"wq{k}") for k in range(MB)]
            wv = [const.tile([P, DIM], f32r, tag=f"wv{k}", name=f"wv{k}") for k in range(MB)]
            bk_sb = const.tile([P, MB], f32, tag="bk")
            bq_sb = const.tile([P, MB], f32, tag="bq")
            bv_sb = const.tile([1, DIM], f32r, tag="bv")
            madd_sb = const.tile([P, nb], f32, tag="madd")
            gamma_sb = const.tile([P, MB], f32, tag="gamma")
            beta_sb = const.tile([P, MB], f32, tag="beta")
            nc.sync.dma_start(out=bk_sb, in_=bk_d[:, :])
            nc.sync.dma_start(out=bq_sb, in_=bq_d[:, :])
            nc.sync.dma_start(out=bv_sb, in_=bv_d[:, :])
            nc.sync.dma_start(out=madd_sb, in_=madd_d[:, :])
            nc.sync.dma_start(out=gamma_sb, in_=gamma_d[:, :])
            nc.sync.dma_start(out=beta_sb, in_=beta_d[:, :])
            ones_row = const.tile([1, P], f32, tag="ones")
            nc.gpsimd.memset(ones_row, 1.0)
            ones_row_r = const.tile([1, P], f32r, tag="onesr")
            nc.sync.dma_start(out=ones_row_r, in_=onesr_d[:, :])
            ones_col = const.tile([P, 1], f32, tag="onesc")
            nc.gpsimd.memset(ones_col, 1.0)
            eps_sb = const.tile([P, 1], f32, tag="eps")
            nc.gpsimd.memset(eps_sb, EPS)
            negs_sb = const.tile([P, 1], f32, tag="negs")
            nc.gpsimd.memset(negs_sb, -float(S))
            sel_sb = const.tile([4, 2], f32, tag="sel")
            nc.sync.dma_start(out=sel_sb, in_=sel_d[:, :])
            for k in range(MB):
                nc.sync.dma_start(out=wk[k], in_=wkT_d[k * P:(k + 1) * P, :])
            for k in range(MB):
                nc.sync.dma_start(out=wq[k], in_=wqT_d[k * P:(k + 1) * P, :])
            for k in range(MB):
                nc.sync.dma_start(out=wv[k], in_=wvT_d[k * P:(k + 1) * P, :])

            def emit_dma(st):
                # issue order tracks the compute critical path:
                # k/v row slots of xa, q columns of xb, mean tail
                xa = [big.tile([P, S], f32r, tag=f"xa{k}", name=f"xa{k}") for k in range(MB)]
                xb = [big.tile([P, S], f32r, tag=f"xb{k}", name=f"xb{k}") for k in range(MB)]
                st["xa"], st["xb"] = xa, xb
                for c0 in range(0, C, 1024):
                    w = min(1024, C - c0)
                    for k in range(MB):
                        nc.sync.dma_start(
                            out=xa[k][:, c0:c0 + w],
                            in_=xaT_d[k * P:(k + 1) * P, c0:c0 + w])
                for c0 in range(0, S, 1024):
                    for k in range(MB):
                        nc.sync.dma_start(
                            out=xb[k][:, c0:c0 + 1024],
                            in_=xbT_d[k * P:(k + 1) * P, c0:c0 + 1024])
                for c0 in range(C, S, 1024):
                    w = min(1024, S - c0)
                    for k in range(MB):
                        nc.sync.dma_start(
                            out=xa[k][:, c0:c0 + w],
                            in_=xaT_d[k * P:(k + 1) * P, c0:c0 + w])

            def emit_kqproj(st):
                # ---- projections k (C slots), q (full S), [d, s] layout ----
                # relu+bias on DVE so ACT stays dedicated to tanh/exp.
                # k/q are stored as fp8e4 [P, 2, cols] (the two d-halves
                # stacked) so the score matmul runs in DoubleRow perf mode:
                # the full 256-deep contraction in one 0.5-cycle/row op.
                # Post-relu values are O(1) so e4m3 (max 240, rel step
                # 2^-4) quantization shifts the dots by well under 1%,
                # and the tanh saturation margin (all dots >= ~15 vs
                # tanh==1.0f cutoff at 9.01) makes that irrelevant.
                # bufs=2 so the next rep's projections overlap this rep's
                # score tail.
                xa, xb = st["xa"], st["xb"]
                kt8 = big.tile([P, 2, C], f8, tag="kt8", bufs=2)
                qt8 = big.tile([P, 2, S], f8, tag="qt8", bufs=2)
                st["kt8"], st["qt8"] = kt8, qt8
                kchunks = [(c0, min(2048, C - c0)) for c0 in range(0, C, 2048)]
                for dst, wsb, bsb, src, chunks in (
                        (kt8, wk, bk_sb, xa, kchunks),
                        (qt8, wq, bq_sb, xb, [(0, 2048), (2048, 2048)])):
                    for m in range(MB):
                        for (c0, w) in chunks:
                            ps = mm_psum.tile([P, w], f32, tag="mm",
                                              name=f"kqps{c0}")
                            for kk in range(MB):
                                for h0 in range(0, w, JC):
                                    hw = min(JC, w - h0)
                                    nc.tensor.matmul(
                                        ps[:, h0:h0 + hw],
                                        lhsT=wsb[kk][:, m * P:(m + 1) * P],
                                        rhs=src[kk][:, c0 + h0:c0 + h0 + hw],
                                        start=(kk == 0), stop=(kk == MB - 1),
                                    )
                            nc.vector.tensor_scalar(
                                out=dst[:, m, c0:c0 + w], in0=ps,
                                scalar1=bsb[:, m:m + 1], scalar2=0.0,
                                op0=OP.add, op1=OP.max)

            def emit_scores(st, rep, hooks):
                # ---- scores + streaming softmax numerator ----
                # s[i] = sum_j tanh(k[i].q[j]); since s <= S the constant
                # shift -S is a safe softmax stabilizer, so each i-block's
                # exp(s - S) and its e*v contribution stream during the
                # score phase (no global max pass, no tail vec matmuls).
                # v-projection (natural [s, d] layout, 2 blocks per psum
                # tile) is interleaved two pairs per i-block so xa frees
                # early for the next rep's DMA without starving ACT.
                xa, kt8, qt8 = st["xa"], st["kt8"], st["qt8"]
                v_sb = big.tile([P, nb * DIM], f32, tag="v")
                s_sb = work.tile([P, nb], f32, tag="s")
                e_sb = work.tile([P, nb], f32, tag="e")
                cs_all = work.tile([P, nb, MB], f32, tag="cs")
                zcols = work.tile([P, ngroups], f32, tag="zc")
                vacc = work.tile([P, DIM], f32, tag="vacc")
                nc.vector.memset(vacc, 0.0)
                xsum = work.tile([P, MB], f32, tag="xsum")
                st["zcols"], st["vacc"], st["xsum"] = zcols, vacc, xsum

                def emit_vpair(pi):
                    ps = mm_psum.tile([P, 2 * DIM], f32, tag="mm", name="vps")
                    for sub in range(2):
                        si = 2 * pi + sub
                        if si >= nb:
                            continue
                        for kk in range(MB):
                            nc.tensor.matmul(
                                ps[:, sub * DIM:(sub + 1) * DIM],
                                lhsT=xa[kk][:, si * P:(si + 1) * P],
                                rhs=wv[kk],
                                start=(kk == 0), stop=False,
                            )
                        nc.tensor.matmul(
                            ps[:, sub * DIM:(sub + 1) * DIM],
                            lhsT=ones_row_r, rhs=bv_sb,
                            start=False, stop=True,
                        )
                    w = min(2 * DIM, (nb - 2 * pi) * DIM)
                    nc.vector.tensor_scalar_max(
                        out=v_sb[:, 2 * pi * DIM:2 * pi * DIM + w],
                        in0=ps[:, :w], scalar1=0.0)

                npairs = (nb + 1) // 2
                for ib in range(nb):
                    hook = hooks.get(ib)
                    if hook is not None:
                        hook()
                    for jg in range(2):  # [128,2048] psum tiles (4 banks)
                        ps = mm_psum.tile([P, 4 * JC], f32, tag="mm",
                                          name=f"scps{jg}")
                        for h in range(4):
                            n = jg * 4 + h
                            nc.tensor.matmul(
                                ps[:, h * JC:(h + 1) * JC],
                                lhsT=kt8[:, :, ib * P:(ib + 1) * P],
                                rhs=qt8[:, :, n * JC:(n + 1) * JC],
                                start=True, stop=True,
                                perf_mode=mybir.MatmulPerfMode.DoubleRow,
                            )
                        # tanh in place in PSUM; row-sum via accum_out
                        nc.scalar.activation(
                            out=ps, in_=ps, func=AF.Tanh,
                            accum_out=cs_all[:, ib, jg:jg + 1],
                        )
                    for t in (2 * ib, 2 * ib + 1):
                        if t < npairs:
                            emit_vpair(t)
                    if ib in ends:
                        g, lo, hi = ends[ib]
                        # masked score columns for the whole group, then
                        # exp and the streamed e*v accumulation
                        nc.vector.reduce_sum(
                            out=s_sb[:, lo:hi], in_=cs_all[:, lo:hi, :],
                            axis=AX.X)
                        nc.vector.tensor_add(
                            s_sb[:, lo:hi], s_sb[:, lo:hi],
                            madd_sb[:, lo:hi])
                        nc.scalar.activation(
                            out=e_sb[:, lo:hi],
                            in_=s_sb[:, lo:hi],
                            func=AF.Exp, bias=negs_sb,
                            accum_out=zcols[:, g:g + 1])
                        for b8 in range(lo, hi):
                            # vacc += e[block] * v[block]
                            nc.vector.scalar_tensor_tensor(
                                out=vacc,
                                in0=v_sb[:, b8 * DIM:(b8 + 1) * DIM],
                                scalar=e_sb[:, b8:b8 + 1], in1=vacc,
                                op0=OP.mult, op1=OP.add)
                        if g == 0:
                            # mean over seq: row-sum of xa, scheduled in
                            # the DVE slack inside the score phase
                            for k in range(MB):
                                nc.vector.reduce_sum(
                                    out=xsum[:, k:k + 1], in_=xa[k],
                                    axis=AX.X)

            def emit_tail(st):
                zcols, vacc, xsum = st["zcols"], st["vacc"], st["xsum"]
                # ---- softmax denominator ----
                zp = work.tile([P, 1], f32, tag="zp")
                nc.vector.reduce_sum(out=zp, in_=zcols, axis=AX.X)
                z_ps = vec_psum.tile([1, 1], f32, tag="mm", name="z_ps")
                nc.tensor.matmul(z_ps, lhsT=zp,
                                 rhs=ones_col, start=True, stop=True)
                z_sb = work.tile([1, 1], f32, tag="z1")
                nc.vector.tensor_copy(out=z_sb, in_=z_ps)
                invz1 = work.tile([1, 1], f32, tag="invz1")
                nc.vector.reciprocal(out=invz1, in_=z_sb)
                invz_ps = vec_psum.tile([P, 1], f32, tag="mm", name="invz_ps")
                nc.tensor.matmul(invz_ps, lhsT=ones_row,
                                 rhs=invz1, start=True, stop=True)
                invz = work.tile([P, 1], f32, tag="invz")
                nc.vector.tensor_copy(out=invz, in_=invz_ps)

                # ---- vec: partition-sum the streamed accumulator ----
                vecp = vec_psum.tile([P, MB], f32, tag="mm", name="vecp")
                for m in range(MB):
                    nc.tensor.matmul(
                        vecp[:, m:m + 1],
                        lhsT=vacc[:, m * P:(m + 1) * P],
                        rhs=ones_col, start=True, stop=True)

                # ---- y = mean(xa) + vec/Z ----
                vscaled = work.tile([P, MB], f32, tag="vs")
                nc.vector.tensor_scalar_mul(out=vscaled, in0=vecp, scalar1=invz)
                # stat4 cols: [y0, y1, y0^2, y1^2]; y_sb aliases cols 0:2
                stat4 = work.tile([P, 4], f32, tag="stat4")
                y_sb = stat4[:, 0:MB]
                nc.vector.scalar_tensor_tensor(
                    out=y_sb, in0=xsum, scalar=1.0 / S, in1=vscaled,
                    op0=OP.mult, op1=OP.add)

                # ---- layernorm over d=256 (spans 2 partition blocks) ----
                nc.vector.tensor_mul(stat4[:, MB:2 * MB], y_sb, y_sb)
                r4_ps = vec_psum.tile([4, 1], f32, tag="mm", name="r4_ps")
                nc.tensor.matmul(r4_ps, lhsT=stat4,
                                 rhs=ones_col, start=True, stop=True)
                r4 = work.tile([4, 1], f32, tag="r4")
                nc.vector.tensor_copy(out=r4, in_=r4_ps)
                s12_ps = vec_psum.tile([1, 2], f32, tag="mm", name="s12_ps")
                nc.tensor.matmul(s12_ps, lhsT=r4,
                                 rhs=sel_sb, start=True, stop=True)
                s12 = work.tile([1, 2], f32, tag="s12")
                nc.vector.tensor_copy(out=s12, in_=s12_ps)
                # mu = sum(y)/D ; ex2 = sum(y^2)/D ; var = ex2 - mu^2
                ms = work.tile([1, 2], f32, tag="ms")
                nc.vector.tensor_scalar_mul(out=ms, in0=s12,
                                            scalar1=1.0 / DIM)
                mu2 = work.tile([1, 1], f32, tag="mu2")
                nc.vector.tensor_mul(mu2, ms[:, 0:1], ms[:, 0:1])
                var = work.tile([1, 1], f32, tag="var")
                nc.vector.tensor_sub(var, ms[:, 1:2], mu2)
                # rstd = exp(-0.5*ln(var+eps))  (ln/exp share a table set)
                lnv = work.tile([1, 1], f32, tag="lnv")
                nc.scalar.activation(out=lnv, in_=var, func=AF.Ln,
                                     bias=eps_sb[0:1, :])
                mr1 = work.tile([1, 2], f32, tag="mr1")
                nc.vector.tensor_copy(out=mr1[:, 0:1], in_=ms[:, 0:1])
                nc.scalar.activation(out=mr1[:, 1:2], in_=lnv, func=AF.Exp,
                                     scale=-0.5)
                # broadcast [mu, rstd] to all partitions
                mr_ps = vec_psum.tile([P, 2], f32, tag="mm", name="mr_ps")
                nc.tensor.matmul(mr_ps, lhsT=ones_row,
                                 rhs=mr1, start=True, stop=True)
                mr_sb = work.tile([P, 2], f32, tag="mr")
                nc.vector.tensor_copy(out=mr_sb, in_=mr_ps)
                # (y - mu) * rstd
                norm = work.tile([P, MB], f32, tag="norm")
                nc.vector.tensor_scalar(
                    out=norm, in0=y_sb, scalar1=mr_sb[:, 0:1],
                    scalar2=mr_sb[:, 1:2], op0=OP.subtract, op1=OP.mult)
                normg = work.tile([P, MB], f32, tag="normg")
                nc.vector.tensor_mul(normg, norm, gamma_sb)
                out_sb = work.tile([P, MB], f32, tag="out")
                nc.vector.tensor_add(out_sb, normg, beta_sb)
                nc.sync.dma_start(out=out_d[:, :], in_=out_sb)

            # ---- software-pipelined emission across reps ----
            # Each engine executes its instructions in issue order, so the
            # next rep's DMA is issued right after the last xa reader
            # (ib 6), its k/q projection mid-score-phase (ib 13, PE/DVE
            # slack), and the serial LN tail of rep r is deferred into rep
            # r+1's score loop (ib 3) — none of them ever block the ACT
            # tanh pipeline, which is the throughput limit.
            states = {0: {}}
            emit_dma(states[0])
            emit_kqproj(states[0])
            for rep in range(reps):
                hooks = {}
                if rep > 0:
                    prev = states.pop(rep - 1)
                    hooks[3] = (lambda p: (lambda: emit_tail(p)))(prev)
                if rep + 1 < reps:
                    nxt = states[rep + 1] = {}
                    hooks[6] = (lambda n: (lambda: emit_dma(n)))(nxt)
                    hooks[13] = (lambda n: (lambda: emit_kqproj(n)))(nxt)
                emit_scores(states[rep], rep, hooks)
            emit_tail(states[reps - 1])

    nc.finalize()
    return nc


def _get_program(reps=1, nb=NB):
    key = (reps, nb)
    if key not in _PROG:
        _PROG[key] = _build_program(reps, nb)
    return _PROG[key]


def _pn(v):
    """[DIM] -> [P, MB] with tile[p, m] = v[m*128 + p]."""
    return np.ascontiguousarray(np.asarray(v, np.float32).reshape(MB, P).T)


def make_in_maps(fingerprint_vectors1, fingerprint_vectors2, mask1, mask2,
                 Wq, bq, Wk, bk, Wv, bv, gamma, beta, nb=NB):
    x1 = np.asarray(fingerprint_vectors1, np.float32)
    x2 = np.asarray(fingerprint_vectors2, np.float32)
    m1 = np.asarray(mask1, bool)
    m2 = np.asarray(mask2, bool)
    x1T = np.ascontiguousarray(x1.transpose(0, 2, 1))  # [B, D, S]
    x2T = np.ascontiguousarray(x2.transpose(0, 2, 1))
    wqT = np.ascontiguousarray(np.asarray(Wq, np.float32).T)
    wkT = np.ascontiguousarray(np.asarray(Wk, np.float32).T)
    wvT = np.ascontiguousarray(np.asarray(Wv, np.float32).T)
    shared = {
        "wkT": wkT, "wqT": wqT, "wvT": wvT,
        "bk": _pn(bk), "bq": _pn(bq),
        "bv": np.ascontiguousarray(np.asarray(bv, np.float32).reshape(1, DIM)),
        "gamma": _pn(gamma), "beta": _pn(beta),
        "sel": np.array([[1, 0], [1, 0], [0, 1], [0, 1]], np.float32),
        "onesr": np.ones((1, P), np.float32),
    }
    in_maps = []
    slots = nb * P
    for b in range(B):
        for stream in range(2):
            if stream == 0:
                xs, xbt, msk = x1[b], x2T[b], m1[b]
            else:
                xs, xbt, msk = x2[b], x1T[b], m2[b]
            # own-stream rows permuted to [unmasked | masked]; only the
            # first `slots` row slots are computed on device, the rest
            # have softmax weight exactly 0
            perm = np.argsort(msk, kind="stable")
            xaP = np.ascontiguousarray(xs[perm].T)
            n_um = int((~msk).sum())
            madd = np.full(slots, np.float32(-1e30), np.float32)
            madd[:min(n_um, slots)] = 0.0
            madd = np.ascontiguousarray(madd.reshape(nb, P).T)
            in_maps.append(dict(shared, xaT=xaP, xbT=xbt, madd=madd))
    return in_maps


# test.py can flip these to get a profile out of the run
RUN_OPTS = {"trace": False, "trace_kwargs": None}
LAST = {}


def kernel(**inputs):
    from concourse.bass_utils import run_bass_kernel_spmd

    m1 = np.asarray(inputs["mask1"], bool)
    m2 = np.asarray(inputs["mask2"], bool)
    n_um_max = max(int((~m1).sum(axis=1).max()), int((~m2).sum(axis=1).max()))
    nb = NB if n_um_max <= NB * P else S // P

    nc = _get_program(1, nb)
    in_maps = make_in_maps(nb=nb, **inputs)
    kw = {}
    if RUN_OPTS.get("trace"):
        kw["trace"] = True
        if RUN_OPTS.get("trace_kwargs"):
            kw["trace_kwargs"] = RUN_OPTS["trace_kwargs"]
    res = run_bass_kernel_spmd(nc, in_maps, list(range(NCORES)), **kw)
    LAST["exec_time_ns"] = res.exec_time_ns
    LAST["profile_json"] = res.profile_json
    outs = res.results
    out1 = np.stack([np.asarray(outs[2 * b]["out"]).T.reshape(DIM)
                     for b in range(B)])
    out2 = np.stack([np.asarray(outs[2 * b + 1]["out"]).T.reshape(DIM)
                     for b in range(B)])
    return out1.astype(np.float32), out2.astype(np.float32)
